# revision 11
# baseline (speedup 1.0000x reference)
"""Multi-head attention (B=2, P=2048, DIM=1024, H=16, d=64) on 8 trn2 cores.

Sharding (v3, pure head-parallel): core c owns heads {2c, 2c+1} for BOTH
batches. Output is sharded (b, q): core c emits rows [512g..512g+512) of
batch b, (b, g) = divmod(c, 4).

Per core:
  - QKV for its 2 heads, both batches, off host-pretransposed bf16 x^T
    (bf16 inputs: rel err ~3e-3, well under the 2e-2 gate; halves DMA).
    Q^T/K^T [128dh, P] per batch (kept fp32 in SBUF); V k-major per head
    with a ones column (softmax-denominator trick).
  - 16 rounds (h, b, qc): S^T tiles [128k, 512q] per 3-chunk group, exp on
    ScalarE (scale 1/8 folded) to bf16, AV matmul with V+ones (denominator
    lands in psum row 64). Normalize via DVE reciprocal + PE ones-matmul
    broadcast (NOT gpsimd partition_broadcast: the pool queue is reserved
    for collectives so in-flight AllToAlls never stall rounds), DVE mul.
  - Two AllToAlls over all 8 cores (one per local head h): slot r of
    cc_in[h] = om(h, b_r, qc=g_r) bf16 — every slot carries real data,
    no cross-batch doubling or zero-padding.
  - Output projection over og [128dh, 8 chunks, 512q] (all real chunks);
    wp = W_proj in bf16, unpermuted (the head->row permutation is the
    identity: 128*(H//2) + 64*(H%2) = 64H), + bias.
"""

import sys

sys.path.insert(0, "/opt/trn_rl_repo")

import numpy as np
import ml_dtypes
import concourse.bass as bass
import concourse.tile as tile
import concourse.mybir as mybir
from concourse import bacc
from concourse.bass import ts
from concourse.bass_utils import run_bass_kernel_spmd

FP = mybir.dt.float32
BF = mybir.dt.bfloat16
N_CORES = 8
B, P, DIM, H, D = 2, 2048, 1024, 16, 64
DHC = 2 * D  # dh per core = 128 (2 heads)
QS = P // 4  # per-core output q-slice = 512
NQ = P // 512  # 4 q-chunks of 512
NK = P // 128  # 16 k-chunks of 128
ND = DIM // 128  # 8 dim-chunks
EXP_GROUP = 3  # k-chunks per exp group (psum tile banks)
QK_DT = mybir.dt.float32r  # S matmul operand dtype (full rate at >=256 rows)
EX_DT = mybir.dt.bfloat16  # exp output / AV operand dtype

_CACHE = {}


def _build(repeat=1, stop_after=None, fake_cc=False):
    nc = bacc.Bacc(
        "TRN2",
        target_bir_lowering=False,
        debug=False,
        enable_asserts=False,
        num_devices=N_CORES,
    )
    # bf16 x^T per batch: [2, DIM, P]; same for all cores.
    xt = nc.dram_tensor("xt", [2, DIM, P], BF, kind="ExternalInput").ap()
    # per-core head-pair slices of W_qkv, [DIM, 128] bf16.
    wq = nc.dram_tensor("wq", [DIM, DHC], BF, kind="ExternalInput").ap()
    wk = nc.dram_tensor("wk", [DIM, DHC], BF, kind="ExternalInput").ap()
    wv = nc.dram_tensor("wv", [DIM, DHC], BF, kind="ExternalInput").ap()
    # W_proj in bf16, unpermuted [DIM, DIM]; bias broadcast [128, DIM].
    wp = nc.dram_tensor("wp", [DIM, DIM], BF, kind="ExternalInput").ap()
    bias = nc.dram_tensor("bias", [128, DIM], FP, kind="ExternalInput").ap()
    out = nc.dram_tensor("out", [QS, DIM], FP, kind="ExternalOutput").ap()

    with tile.TileContext(nc) as tc:
        with (
            tc.tile_pool(name="s1", bufs=1) as s1,
            tc.tile_pool(name="es", bufs=7) as es,
            tc.tile_pool(name="wk2", bufs=2) as wk2,
            tc.tile_pool(name="dram", bufs=1, space="DRAM") as dram,
            tc.tile_pool(name="spool", bufs=2, space="PSUM") as spool,
            tc.tile_pool(name="avpool", bufs=2, space="PSUM") as avpool,
        ):
            # persistent SBUF
            qt_s = s1.tile([128, 2, P], QK_DT)  # Q^T per batch
            kt_s = s1.tile([128, 2, NK, 128], QK_DT)  # K^T k-chunked
            v_s = s1.tile([128, 2, NK, 2, D + 1], EX_DT)  # V k-major + ones
            bias_s = s1.tile([128, DIM], FP)
            ones_s = s1.tile([1, D], QK_DT)
            nc.sync.dma_start(bias_s[:], bias[:])
            ones_f = s1.tile([1, D], FP)
            nc.vector.memset(v_s[:, :, :, :, D : D + 1], 1.0)
            nc.vector.memset(ones_f[:], 1.0)
            nc.vector.tensor_copy(out=ones_s[:], in_=ones_f[:])

            # A2A buffers: slot r of cc_in[h] = om(h, b_r, qc=g_r), bf16
            cc_in = [dram.tile([8, D, QS], EX_DT, name=f"cci{h}") for h in range(2)]
            cc_out = [dram.tile([8, D, QS], EX_DT, name=f"cco{h}") for h in range(2)]

            def one_pass():
              with tc.tile_pool(name="ld", bufs=1) as ld:
                xt_s = ld.tile([128, ND, P], BF)  # one batch at a time
                wq_s = ld.tile([128, ND, DHC], BF)
                wk_s = ld.tile([128, ND, DHC], BF)
                wv_s = ld.tile([128, ND, DHC], BF)

                def load_xt(b, qc):
                    # one batched DMA per (b, qc): [1024, 512] -> [128, 8, 512]
                    nc.sync.dma_start(
                        xt_s[:, :, ts(qc, 512)],
                        xt[b, :, ts(qc, 512)].rearrange("(c p) n -> p c n", p=128),
                    )

                nc.sync.dma_start(
                    wq_s[:], wq.rearrange("(c p) n -> p c n", p=128)
                )
                load_xt(0, 0)
                nc.sync.dma_start(
                    wk_s[:], wk.rearrange("(c p) n -> p c n", p=128)
                )
                nc.sync.dma_start(
                    wv_s[:], wv.rearrange("(c p) n -> p c n", p=128)
                )
                for qc in range(1, NQ):
                    load_xt(0, qc)

                def qk_chunk(b, qc):
                    psq = spool.tile([128, EXP_GROUP, 512], FP, tag="st", name="psq")
                    psk = spool.tile([128, EXP_GROUP, 512], FP, tag="st", name="psk")
                    for dc in range(ND):
                        nc.tensor.matmul(
                            psq[:, 0, :],
                            wq_s[:, dc, :],
                            xt_s[:, dc, ts(qc, 512)],
                            start=(dc == 0),
                            stop=(dc == ND - 1),
                        )
                    for dc in range(ND):
                        nc.tensor.matmul(
                            psk[:, 0, :],
                            wk_s[:, dc, :],
                            xt_s[:, dc, ts(qc, 512)],
                            start=(dc == 0),
                            stop=(dc == ND - 1),
                        )
                    nc.vector.tensor_copy(
                        out=qt_s[:, b, ts(qc, 512)], in_=psq[:, 0, :]
                    )
                    nc.vector.tensor_copy(
                        out=kt_s[:, b, 4 * qc : 4 * qc + 4, :],
                        in_=psk[:, 0, :].rearrange("p (a c) -> p a c", c=128),
                    )

                def v_chunk(b, sc):
                    psv = spool.tile([128, EXP_GROUP, 512], FP, tag="st", name="psv")
                    for dc in range(ND):
                        nc.tensor.matmul(
                            psv[:, 0, 0:DHC],
                            xt_s[:, dc, ts(sc, 128)],
                            wv_s[:, dc, :],
                            start=(dc == 0),
                            stop=(dc == ND - 1),
                        )
                    nc.vector.tensor_copy(
                        out=v_s[:, b, sc, :, 0:D],
                        in_=psv[:, 0, 0:DHC].rearrange("p (h d) -> p h d", d=D),
                    )

                # QKV both batches; xt_s reloaded for b=1 (subtile WAR)
                for b in range(2):
                    if b == 1:
                        for qc in range(NQ):
                            load_xt(1, qc)
                    for qc in range(NQ):
                        qk_chunk(b, qc)
                        for sc in range(4 * qc, 4 * qc + 4):
                            v_chunk(b, sc)

              if stop_after == "qkv":
                  nc.sync.dma_start(out[0:128, 0:512], qt_s[:, 0, 0:512].bitcast(FP))
                  return

              # ---- rounds + proj (s2 reuses ld's sbuf range) --------------
              with tc.tile_pool(name="s2", bufs=1) as s2:
                wp_s = s2.tile([128, ND, DIM], BF)
                og_s = s2.tile([128, ND, QS], BF)
                obuf = s2.tile([128, 8, 512], FP)
                nc.sync.dma_start(wp_s[:], wp.rearrange("(c p) n -> p c n", p=128))

                def og_dma(h):
                    nc.sync.dma_start(
                        og_s[64 * h : 64 * h + 64, :, :],
                        cc_out[h].rearrange("s p n -> p s n"),
                    )

                def emit_cc(h):
                    if fake_cc:
                        nc.sync.dma_start(cc_out[h][:], cc_in[h][:])
                    else:
                        nc.gpsimd.collective_compute(
                            "AllToAll",
                            mybir.AluOpType.bypass,
                            replica_groups=[list(range(N_CORES))],
                            ins=[cc_in[h].opt()],
                            outs=[cc_out[h].opt()],
                        )
                    og_dma(h)

                om_cur = [None]

                def emit_tail(h, b, qc, av):
                    rec = wk2.tile([1, 512], QK_DT, tag="rec", name="rec")
                    with nc.allow_low_precision(reason="fp32r reciprocal for bc"):
                        nc.vector.reciprocal(rec[:], av[D : D + 1, :])
                    bc = spool.tile([128, EXP_GROUP, 512], FP, tag="st", name="bc")
                    nc.tensor.matmul(
                        bc[0:D, 0, :], ones_s[:], rec[:], start=True, stop=True
                    )
                    bcs = wk2.tile([D, 512], FP, tag="bcs", name="bcs")
                    nc.vector.tensor_copy(out=bcs[:], in_=bc[0:D, 0, :])
                    if qc == 0:
                        om_cur[0] = wk2.tile([D, NQ, 512], EX_DT, tag="om", name="om")
                    om = om_cur[0]
                    nc.vector.tensor_mul(om[:, qc, :], av[0:D, :], bcs[:])
                    if qc == NQ - 1:
                        # one batched DMA per (h, b): 4 slots at once
                        nc.sync.dma_start(
                            cc_in[h][4 * b : 4 * b + 4, :, :].rearrange(
                                "s p n -> p s n"
                            ),
                            om[:],
                        )

                import collections as _c

                pend = _c.deque()  # (h, b, av, ex, k0, k1, tail|None)

                def flush_one():
                    h_, b_, av_, ex_, k0_, k1_, tinfo = pend.popleft()
                    for k in range(k0_, k1_):
                        nc.tensor.matmul(
                            av_[0 : D + 1, :],
                            v_s[:, b_, k, h_, :],
                            ex_[:, k - k0_, :],
                            start=(k == 0),
                            stop=(k == NK - 1),
                            skip_group_check=True,
                        )
                    if tinfo is not None:
                        th, tb, tqc = tinfo
                        emit_tail(th, tb, tqc, av_)
                        if th == 0 and tb == 1 and tqc == NQ - 1:
                            emit_cc(0)

                av_cur = [None]
                groups = [
                    (k0, min(k0 + EXP_GROUP, NK)) for k0 in range(0, NK, EXP_GROUP)
                ]

                def round_groups(h, b, qc):
                    hp = 64 * h
                    for gi, (k0, k1) in enumerate(groups):
                        st = spool.tile(
                            [128, EXP_GROUP, 512], FP, tag="st", name="st"
                        )
                        for k in range(k0, k1):
                            nc.tensor.matmul(
                                st[:, k - k0, :],
                                kt_s[hp : hp + 64, b, k, :],
                                qt_s[hp : hp + 64, b, ts(qc, 512)],
                                start=True,
                                stop=True,
                            )
                        ex = es.tile(
                            [128, EXP_GROUP, 512], EX_DT, tag="ex", name="ex"
                        )
                        nc.scalar.activation(
                            out=ex[:, 0 : k1 - k0, :],
                            in_=st[:, 0 : k1 - k0, :],
                            func=mybir.ActivationFunctionType.Exp,
                            scale=float(D) ** -0.5,
                        )
                        if gi == 0:
                            av_cur[0] = avpool.tile(
                                [128, 512], FP, tag="av", name="av"
                            )
                        pend.append(
                            (
                                h,
                                b,
                                av_cur[0],
                                ex,
                                k0,
                                k1,
                                (h, b, qc) if gi == len(groups) - 1 else None,
                            )
                        )
                        while len(pend) > 2:
                            flush_one()

                for h in range(2):
                    for b in range(2):
                        for qc in range(NQ):
                            round_groups(h, b, qc)
                while pend:
                    flush_one()
                if stop_after == "rounds":
                    nc.sync.dma_start(
                        out[0:64, 0:256].bitcast(EX_DT), cc_in[0][0, :, :]
                    )
                    return
                emit_cc(1)

                def proj_pass(u):
                    oc, sc = divmod(u, 4)
                    pso = spool.tile(
                        [128, EXP_GROUP, 512], FP, tag="st", name="pso"
                    )
                    for c in range(ND):
                        nc.tensor.matmul(
                            pso[:, 0, :],
                            og_s[:, c, ts(sc, 128)],
                            wp_s[:, c, ts(oc, 512)],
                            start=(c == 0),
                            stop=(c == ND - 1),
                        )
                    nc.vector.tensor_add(
                        obuf[:, u, :], pso[:, 0, :], bias_s[:, ts(oc, 512)]
                    )
                    if sc == 3:
                        # one batched DMA per oc half: [512, 512]
                        nc.sync.dma_start(
                            out[:, ts(oc, 512)].rearrange("(s p) n -> p s n", p=128),
                            obuf[:, 4 * oc : 4 * oc + 4, :],
                        )

                for u in range(8):
                    proj_pass(u)

            for _rep in range(repeat):
                one_pass()

    nc.compile()
    return nc


def _prep_inputs(x, W_qkv, W_proj, b_proj):
    """Host-side prep: per-core input dicts (bf16 x and weights)."""
    bf = ml_dtypes.bfloat16
    x = np.asarray(x, dtype=np.float32)
    W_qkv = np.asarray(W_qkv, dtype=np.float32)
    W_proj = np.asarray(W_proj, dtype=np.float32)
    b_proj = np.asarray(b_proj, dtype=np.float32)

    xt = np.ascontiguousarray(np.transpose(x, (0, 2, 1)).astype(bf))  # [2, DIM, P]
    wp_bf = np.ascontiguousarray(W_proj.astype(bf))
    bias_b = np.ascontiguousarray(np.broadcast_to(b_proj[None, :], (128, DIM)))
    in_maps = []
    for c in range(N_CORES):
        lo, hi = DHC * c, DHC * (c + 1)
        in_maps.append(
            {
                "xt": xt,
                "wq": np.ascontiguousarray(W_qkv[:, lo:hi].astype(bf)),
                "wk": np.ascontiguousarray(W_qkv[:, DIM + lo : DIM + hi].astype(bf)),
                "wv": np.ascontiguousarray(
                    W_qkv[:, 2 * DIM + lo : 2 * DIM + hi].astype(bf)
                ),
                "wp": wp_bf,
                "bias": bias_b,
            }
        )
    return in_maps


def kernel(x, W_qkv, W_proj, b_proj, _trace=False, _tmpdir=None):
    if "nc" not in _CACHE:
        _CACHE["nc"] = _build()
    nc = _CACHE["nc"]
    in_maps = _prep_inputs(x, W_qkv, W_proj, b_proj)
    res = run_bass_kernel_spmd(
        nc,
        in_maps,
        core_ids=list(range(N_CORES)),
        trace=_trace,
        tmpdir=_tmpdir,
        stitch_traces=False,
    )
    _CACHE["last_results"] = res
    full = np.empty((B, P, DIM), dtype=np.float32)
    for c in range(N_CORES):
        b, g = divmod(c, 4)
        full[b, QS * g : QS * (g + 1), :] = res.results[c]["out"]
    return full
